# revision 56
# baseline (speedup 1.0000x reference)
"""Deformable Conv2d Trainium kernel: host prep + bass program builder.

Sharding: 8 cores = 4 batches x 2 height-halves; each core computes
out[b, :, h0:h0+32, :] (128 out-ch x 8192 positions).

Position layout per core: pos = jj*128 + p, jj in [0,64), p in [0,128);
ho_local = jj//2, wo = (jj%2)*128 + p.

Device pipeline (per-quarter software pipeline; the Q7 dma_gather
descriptor-generation stream [~16.4us per 2048-idx gather] is the
bottleneck, so all prep (B-E) for quarter Q+1 and all consume work
(DVE weighting, PE transposes/matmuls, output DMA) for quarter Q-1
must hide underneath quarter Q's 9 gathers):
  per quarter Q (2048 positions = 16 jj blocks):
    B: offset conv om_q[27, 2048] (PE, fp16 inputs, fp32 accum)
    C: omT via PE transposes -> omt_q [128, 16, 27] f32
    D: bilinear weight math (DVE/ACT) on [128, 144] slices
       -> V4 [128, 64, 9, 4] fp16, varf_q [128,144] f32
    E: idx fold via 8 permutation matmuls -> IDXG [128, 4, 9, 16, 8] int16
    F: per tap k: dma_gather (DRAM table, 512B items) -> G [128,16,256] fp16;
       H = G*V4 (DVE); corner-reduce -> SAMPT;
       PE transposes (batched 4-jl into [128,512] f16 PSUM) -> RHS [128,512];
       weight-stationary matmuls (5 ldw per 512-pos block) -> PSUM -> OUT
"""
import numpy as np

import concourse.bass as bass
import concourse.mybir as mybir
import concourse.tile as tile
from concourse import bacc

F32 = mybir.dt.float32
F16 = mybir.dt.float16
I16 = mybir.dt.int16
I32 = mybir.dt.int32

C = 64
O = 128
H = 64
W = 256
KK = 9
BD = 4                 # vertical halo margin (max |dy| must be < BD)
MX = 5                 # horizontal margin (max |dx| must be < MX)
RT = 42                # table rows: covers y0 in [h0-1-BD, h0+32+BD] inclusive
PITCH = 384            # table row pitch (multiple of 128, >= 270)
TCW = 268              # valid table cols: tc = x0 + MX + 1 in [0, 267]
XCW = TCW + 1          # padded x-slice cols (item needs tc+1)
NIT = RT * PITCH       # table items (16128)
NPOS = 32 * W          # 8192 positions per core
NJJ = 64               # pos blocks of 128
NQ = 4                 # quarters
JQ = 16                # jj per quarter
NF = NJJ * KK          # 576
QF = JQ * KK           # 144 free elems per quarter in (jj,k) layout


def _xpad_slice(xb, h0):
    """[C, RT, XCW] zero-padded fp16 slice; rows y_base..y_base+RT-1, col tc=xg+MX+1."""
    y_base = h0 - 1 - BD
    xp = np.zeros((C, RT, XCW), np.float16)
    r0 = max(0, -y_base)
    r1 = min(RT, H - y_base)
    xp[:, r0:r1, MX + 1:MX + 1 + W] = xb[:, y_base + r0:y_base + r1, :].astype(np.float16)
    return xp


def _xp2(xp):
    """[128, RT, XCW]: partitions 0:64 = xp, 64:128 = xp shifted one row up
    (row r holds xp row r+1), so a 128-partition contraction covers vertical
    tap pairs (ki=0,1) in one matmul pass."""
    sh = np.zeros_like(xp)
    sh[:, :RT - 1] = xp[:, 1:]
    return np.concatenate([xp, sh], axis=0)


def _table(xp):
    """[NIT, 256] fp16 gather table; item (r, tc) = per-c [v00, v01, v10, v11]."""
    it = np.zeros((RT, PITCH, C, 4), np.float16)
    it[:RT - 1, :TCW, :, 0] = xp[:, :RT - 1, :TCW].transpose(1, 2, 0)
    it[:RT - 1, :TCW, :, 1] = xp[:, :RT - 1, 1:TCW + 1].transpose(1, 2, 0)
    it[:RT - 1, :TCW, :, 2] = xp[:, 1:RT, :TCW].transpose(1, 2, 0)
    it[:RT - 1, :TCW, :, 3] = xp[:, 1:RT, 1:TCW + 1].transpose(1, 2, 0)
    return it.reshape(NIT, 256)


def host_prep(x, offset_weight, offset_bias, weight):
    """Returns list of 8 in_map dicts; core order = (b, hh)."""
    jj = np.arange(NJJ)
    kv = np.arange(KK)
    ki = kv // 3
    kj = kv % 3
    p = np.arange(128)

    # vertical-pair offset-conv weights: row u*64+c of pass kj = w[o, c, ki=u, kj]
    owr = offset_weight.reshape(27, C, 3, 3)
    ow2p2 = np.zeros((128, 3, 27), np.float16)
    for u in range(2):
        for q in range(3):
            ow2p2[u * 64:(u + 1) * 64, q, :] = owr[:, :, u, q].T
    ow2s2 = np.zeros((C, 3, 27), np.float16)
    for q in range(3):
        ow2s2[:, q, :] = owr[:, :, 2, q].T
    ob = offset_bias.reshape(27, 1).astype(np.float32)
    w2 = weight.reshape(O, C, KK)
    w2p = np.zeros((128, 4, 128), np.float16)
    for t in range(4):
        w2p[:64, t, :] = w2[:, :, 2 * t].T.astype(np.float16)
        w2p[64:, t, :] = w2[:, :, 2 * t + 1].T.astype(np.float16)
    w2s = np.ascontiguousarray(w2[:, :, 8].T.astype(np.float16))          # [64, 128]

    x0b = ((jj[None, :, None] % 2) * 128 + p[:, None, None] + kj[None, None, :] - 1
           ).astype(np.float32).reshape(128, NF)
    base = ((jj[None, :, None] // 2 + BD + ki[None, None, :]) * PITCH
            + (jj[None, :, None] % 2) * 128 + p[:, None, None]
            + kj[None, None, :] + MX).astype(np.float32)                   # [128, 64, 9]
    # BASEG [128, NQ, KK, JQ, 8] f32: [rr, Q, k, j, q] = base[q*16+rr, Q*16+j, k]
    baseg = np.zeros((16, NQ, KK, JQ, 8), np.float32)
    for q in range(8):
        for rr in range(16):
            b16 = base[q * 16 + rr]                      # [64, 9]
            baseg[rr, :, :, :, q] = b16.reshape(NQ, JQ, KK).transpose(0, 2, 1)
    baseg = np.tile(baseg, (8, 1, 1, 1, 1)).reshape(128, NQ * KK * JQ * 8)

    # E-phase fold matrices: matmul q maps varf partitions q*16..q*16+15 to a
    # [32, nf] PSUM tile with rows 16-31 duplicating rows 0-15, so the idx add
    # writes partitions 0-31 directly (the gather ucode reads exactly 32) and
    # no broadcast DMA is needed.
    permq = np.zeros((128, 8, 32), dtype=np.float32)
    for q in range(8):
        for rr in range(16):
            permq[q * 16 + rr, q, rr] = 1.0
            permq[q * 16 + rr, q, 16 + rr] = 1.0
    permq = permq.reshape(128, 256)
    idf16 = np.eye(128, dtype=np.float16)
    idf32 = np.eye(27, dtype=np.float32)
    # per-partition constants for 2-input DVE ops (1-input tensor_scalar/copy
    # ops run in DVE 2-port mode, which contends with the Q7 gather
    # descriptor-gen for SBUF ports and stalls up to 12us mid-stream):
    # cols: 0, 1, -1, H-1, H-2, W-1, W-2, round-magic (1.5*2^23)
    cb = np.tile(np.array([0.0, 1.0, -1.0, H - 1, H - 2, W - 1, W - 2,
                           12582912.0], np.float32), (128, 1))

    in_maps = []
    for core in range(8):
        b, hh = core // 2, core % 2
        h0 = hh * 32
        xp = _xpad_slice(x[b], h0)
        hoky = ((h0 + jj[None, :, None] // 2 + ki[None, None, :] - 1)
                * np.ones((128, 1, 1))).astype(np.float32)
        in_maps.append({
            "XP2": np.ascontiguousarray(_xp2(xp).reshape(128, RT * XCW)),
            "TBL": np.ascontiguousarray(_table(xp)),
            "OW2P2": np.ascontiguousarray(ow2p2.reshape(128, 3 * 27)),
            "OW2S2": np.ascontiguousarray(ow2s2.reshape(C, 3 * 27)),
            "OB": ob,
            "W2P": w2p, "W2S": w2s,
            "HOKY": np.ascontiguousarray(hoky.reshape(128, NF)), "X0B": x0b,
            "BASEG": baseg, "PERMQ": permq, "IDF16": idf16, "IDF32": idf32,
            "CB": cb,
        })
    return in_maps


def host_post(outs):
    """outs: list of 8 [128, 8192] f32 -> [4, 128, 64, 256]."""
    y = np.zeros((4, O, H, W), np.float32)
    for core, o in enumerate(outs):
        b, hh = core // 2, core % 2
        v = np.asarray(o).reshape(O, 32, 2, 128).reshape(O, 32, 256)
        y[b, :, hh * 32:hh * 32 + 32, :] = v
    return y


def _bcast(ap, dim, n):
    """Insert a [0, n] broadcast dim at free position `dim` (1-based in ap list)."""
    newap = [list(d) for d in ap.ap]
    newap.insert(dim, [0, n])
    return bass.AP(tensor=ap.tensor, offset=ap.offset, ap=newap)


def build(nc=None):
    if nc is None:
        nc = bacc.Bacc("TRN2", target_bir_lowering=False, debug=False)
    XP2 = nc.dram_tensor("XP2", [128, RT * XCW], F16, kind="ExternalInput")
    TBL = nc.dram_tensor("TBL", [NIT, 256], F16, kind="ExternalInput")
    OW2P2 = nc.dram_tensor("OW2P2", [128, 3 * 27], F16, kind="ExternalInput")
    OW2S2 = nc.dram_tensor("OW2S2", [C, 3 * 27], F16, kind="ExternalInput")
    OB = nc.dram_tensor("OB", [27, 1], F32, kind="ExternalInput")
    W2P = nc.dram_tensor("W2P", [128, 4, 128], F16, kind="ExternalInput")
    W2S = nc.dram_tensor("W2S", [C, 128], F16, kind="ExternalInput")
    HOKY = nc.dram_tensor("HOKY", [128, NF], F32, kind="ExternalInput")
    X0B = nc.dram_tensor("X0B", [128, NF], F32, kind="ExternalInput")
    BASEG = nc.dram_tensor("BASEG", [128, NQ * KK * JQ * 8], F32, kind="ExternalInput")
    PERMQ = nc.dram_tensor("PERMQ", [128, 8 * 32], F32, kind="ExternalInput")
    IDF16 = nc.dram_tensor("IDF16", [128, 128], F16, kind="ExternalInput")
    IDF32 = nc.dram_tensor("IDF32", [27, 27], F32, kind="ExternalInput")
    CB = nc.dram_tensor("CB", [128, 8], F32, kind="ExternalInput")
    OUT = nc.dram_tensor("OUT", [O, NPOS], F32, kind="ExternalOutput")

    mm = mybir.AluOpType

    with tile.TileContext(nc) as tc:
        with (
            tc.tile_pool(name="const", bufs=1) as cpool,
            tc.tile_pool(name="wq", bufs=1) as wq,
            tc.tile_pool(name="wq2", bufs=2) as wq2,
            tc.tile_pool(name="gat", bufs=3) as gpool,
            tc.tile_pool(name="hb", bufs=2) as hpool,
            tc.tile_pool(name="samp", bufs=2) as spool,
            tc.tile_pool(name="rhs", bufs=12) as rpool,
            tc.tile_pool(name="rhss", bufs=5) as rspool,
            tc.tile_pool(name="oq", bufs=2) as opool,
            tc.tile_pool(name="psB", bufs=1, space="PSUM") as psB,
            tc.tile_pool(name="psP", bufs=1, space="PSUM") as psP,
            tc.tile_pool(name="psTp", bufs=2, space="PSUM") as psTp,
            tc.tile_pool(name="psO", bufs=4, space="PSUM") as psO,
        ):
            # ---------- constants, ordered by when the serial prep(0) chain
            # needs them: tiny offset-conv weights, then just enough xp2 rows
            # for the first slabs' conv; the bulky baseg/xp2-tail/w2p queue
            # after everything that gates the first gather ----------
            ow2p2 = cpool.tile([128, 3, 27], F16)
            nc.sync.dma_start(out=ow2p2,
                              in_=OW2P2[:, :].rearrange("c (k o) -> c k o", k=3))
            ow2s2 = cpool.tile([C, 3, 27], F16)
            nc.sync.dma_start(out=ow2s2,
                              in_=OW2S2[:, :].rearrange("c (k o) -> c k o", k=3))
            ob = cpool.tile([27, 1], F32)
            nc.sync.dma_start(out=ob, in_=OB[:, :])
            idf32 = cpool.tile([27, 27], F32)
            nc.sync.dma_start(out=idf32, in_=IDF32[:, :])
            cb = cpool.tile([128, 8], F32)
            nc.sync.dma_start(out=cb, in_=CB[:, :])
            xp2 = cpool.tile([128, RT, XCW], F16)
            nc.sync.dma_start(out=xp2[:, 0:16, :],
                              in_=XP2[:, 0:16 * XCW].rearrange(
                                  "c (r w) -> c r w", r=16))

            def cbv(col, n, p=128):
                """[p, n] broadcast AP of constant column `col` of cb."""
                a = cb[0:p, col:col + 1]
                return bass.AP(tensor=a.tensor, offset=a.offset,
                               ap=[list(a.ap[0]), [0, n]])
            hoky = cpool.tile([128, NF], F32)
            nc.sync.dma_start(out=hoky, in_=HOKY[:, :])
            x0b = cpool.tile([128, NF], F32)
            nc.sync.dma_start(out=x0b, in_=X0B[:, :])
            permq = cpool.tile([128, 8, 32], F32)
            nc.sync.dma_start(out=permq,
                              in_=PERMQ[:, :].rearrange("c (q m) -> c q m", q=8))
            baseg = cpool.tile([128, NQ, KK, JQ, 8], F32)
            nc.sync.dma_start(out=baseg,
                              in_=BASEG[:, :].rearrange(
                                  "r (a k j q) -> r a k j q", a=NQ, k=KK, j=JQ))
            nc.sync.dma_start(out=xp2[:, 16:RT, :],
                              in_=XP2[:, 16 * XCW:].rearrange(
                                  "c (r w) -> c r w", r=RT - 16))
            w2p = cpool.tile([128, 4, 128], F16)
            nc.sync.dma_start(out=w2p, in_=W2P[:, :, :])
            w2s = cpool.tile([C, 128], F16)
            nc.sync.dma_start(out=w2s, in_=W2S[:, :])
            idf16 = cpool.tile([128, 128], F16)
            nc.sync.dma_start(out=idf16, in_=IDF16[:, :])

            def prep_quarter(jj0, njj):
                # high_priority: the Tile scheduler otherwise places this
                # slab's idx-add AFTER the previous slab's h/s2/s3 chain in
                # the DVE stream, making the next slab's first gather wait
                # ~5-7us at every boundary for the previous slab's last
                # gather data to flow through the DVE.
                with tc.high_priority():
                    return _prep_quarter(jj0, njj)

            def _prep_quarter(jj0, njj):
                """Phases B-E for a slab of njj jj-blocks starting at jj0.

                Returns (v4_q, idxg_q) double-buffered per-slab tiles so the
                next slab's prep never WAR-stalls on this slab's gathers.
                The first quarter runs as two 8-jj slabs to halve the serial
                prep latency ahead of the first gather."""
                nf = njj * KK
                s = slice(jj0 * KK, (jj0 + njj) * KK)   # slice in (jj,k) layout
                Q = jj0 // JQ
                j0l = jj0 % JQ

                # ---- B: offset conv (vertical tap pairs; 6 passes) ----
                om_q = wq.tile([27, njj * 128], F32, tag="om")
                for cc in range(njj // 4):
                    ch = jj0 // 4 + cc
                    ps = psB.tile([27, 512], F32, tag="psom")
                    r = 2 * ch + BD
                    for kj in range(3):
                        rhs = xp2[:, r:r + 2, kj + MX:kj + MX + W]
                        nc.tensor.matmul(ps, ow2p2[:, kj, :], rhs,
                                         start=(kj == 0), stop=False)
                    for kj in range(3):
                        rhs = xp2[0:C, r + 2:r + 4, kj + MX:kj + MX + W]
                        nc.tensor.matmul(ps, ow2s2[:, kj, :], rhs,
                                         start=False, stop=(kj == 2))
                    nc.vector.scalar_tensor_tensor(
                        out=om_q[:, cc * 512:(cc + 1) * 512], in0=ps,
                        scalar=ob, in1=cbv(0, 512, p=27),
                        op0=mm.add, op1=mm.add)

                # ---- C: transpose om -> omt_q [128, 16, 27] ----
                omt_q = wq.tile([128, njj, 27], F32, tag="omt")
                for j in range(njj):
                    pst = psP.tile([128, 27], F32, tag="prep_ps")
                    nc.tensor.transpose(pst, om_q[:, j * 128:(j + 1) * 128], idf32)
                    nc.scalar.copy(omt_q[:, j, :], pst)

                # ---- D: weight math on [128, 144] ----
                DY = omt_q[:, :, 0:18:2]
                DX = omt_q[:, :, 1:18:2]
                MZ = omt_q[:, :, 18:27]

                msig = wq.tile([128, nf], F32, tag="msig")
                nc.scalar.activation(out=msig, in_=MZ,
                                     func=mybir.ActivationFunctionType.Sigmoid)

                def floor_frac(src_ap, ftag):
                    # floor via fp32 magic rounding (t = (src+1.5*2^23)-1.5*2^23
                    # rounds to integer; then f = t - (src < t)). All 2-input
                    # TT ops: no DVE 2-port mode, no Q7 port contention.
                    tm = wq.tile([128, nf], F32, tag="flr_m")
                    nc.vector.tensor_tensor(out=tm, in0=src_ap,
                                            in1=cbv(7, nf), op=mm.add)
                    tf = wq.tile([128, nf], F32, tag="flr_f")
                    nc.vector.tensor_tensor(out=tf, in0=tm,
                                            in1=cbv(7, nf), op=mm.subtract)
                    lt = wq.tile([128, nf], F32, tag="flr_lt")
                    nc.vector.tensor_tensor(out=lt, in0=src_ap, in1=tf,
                                            op=mm.is_lt)
                    fl = wq.tile([128, nf], F32, tag=ftag + "_f")
                    nc.vector.tensor_tensor(out=fl, in0=tf, in1=lt,
                                            op=mm.subtract)
                    fr = wq.tile([128, nf], F32, tag=ftag + "_l")
                    nc.vector.tensor_tensor(out=fr, in0=src_ap, in1=fl,
                                            op=mm.subtract)
                    return fl, fr

                fy, ly = floor_frac(DY, "fy")
                fx, lx = floor_frac(DX, "fx")

                y0 = wq.tile([128, nf], F32, tag="y0")
                nc.vector.tensor_tensor(out=y0, in0=fy, in1=hoky[:, s], op=mm.add)
                x0 = wq.tile([128, nf], F32, tag="x0")
                nc.vector.tensor_tensor(out=x0, in0=fx, in1=x0b[:, s], op=mm.add)

                def in_range(src, lo_col, hi_col, out_tag):
                    a = wq.tile([128, nf], F32, tag="rng_a")
                    nc.vector.tensor_tensor(out=a, in0=src,
                                            in1=cbv(lo_col, nf), op=mm.is_ge)
                    bq = wq.tile([128, nf], F32, tag="rng_b")
                    nc.vector.tensor_tensor(out=bq, in0=src,
                                            in1=cbv(hi_col, nf), op=mm.is_le)
                    rr = wq.tile([128, nf], F32, tag=out_tag)
                    nc.vector.tensor_tensor(out=rr, in0=a, in1=bq, op=mm.mult)
                    return rr

                vy0 = in_range(y0, 0, 3, "vy0")
                vy1 = in_range(y0, 2, 4, "vy1")
                vx0 = in_range(x0, 0, 5, "vx0")
                vx1 = in_range(x0, 2, 6, "vx1")

                ily = wq.tile([128, nf], F32, tag="ily")
                nc.vector.scalar_tensor_tensor(out=ily, in0=ly, scalar=-1.0,
                                               in1=cbv(1, nf),
                                               op0=mm.mult, op1=mm.add)
                ilx = wq.tile([128, nf], F32, tag="ilx")
                nc.vector.scalar_tensor_tensor(out=ilx, in0=lx, scalar=-1.0,
                                               in1=cbv(1, nf),
                                               op0=mm.mult, op1=mm.add)

                a0 = wq.tile([128, nf], F32, tag="a0")
                nc.vector.tensor_tensor(out=a0, in0=msig, in1=ily, op=mm.mult)
                nc.vector.tensor_tensor(out=a0, in0=a0, in1=vy0, op=mm.mult)
                a1 = wq.tile([128, nf], F32, tag="a1")
                nc.vector.tensor_tensor(out=a1, in0=msig, in1=ly, op=mm.mult)
                nc.vector.tensor_tensor(out=a1, in0=a1, in1=vy1, op=mm.mult)
                b0 = wq.tile([128, nf], F32, tag="b0")
                nc.vector.tensor_tensor(out=b0, in0=ilx, in1=vx0, op=mm.mult)
                b1 = wq.tile([128, nf], F32, tag="b1")
                nc.vector.tensor_tensor(out=b1, in0=lx, in1=vx1, op=mm.mult)

                v4_q = wq2.tile([128, njj, KK, 4], F16, tag="v4")
                nc.vector.tensor_tensor(out=v4_q[:, :, :, 0], in0=a0, in1=b0, op=mm.mult)
                nc.vector.tensor_tensor(out=v4_q[:, :, :, 1], in0=a0, in1=b1, op=mm.mult)
                nc.vector.tensor_tensor(out=v4_q[:, :, :, 2], in0=a1, in1=b0, op=mm.mult)
                nc.vector.tensor_tensor(out=v4_q[:, :, :, 3], in0=a1, in1=b1, op=mm.mult)

                varf = wq.tile([128, nf], F32, tag="varf")
                nc.vector.scalar_tensor_tensor(out=varf, in0=fy, scalar=float(PITCH),
                                               in1=fx, op0=mm.mult, op1=mm.add)

                # ---- E: idx fold (8 perm matmuls -> varg [32,(k,j,q)]).
                # permq duplicates rows into partitions 16-31, covering both
                # Q7 core idx reads with no broadcast DMA. ----
                varg = wq.tile([32, KK, njj, 8], F32, tag="varg")
                for q in range(8):
                    psf = psP.tile([32, nf], F32, tag="prep_ps")
                    nc.tensor.matmul(psf, permq[:, q, :],
                                     varf[:, :], start=True, stop=True)
                    # psf free = (j, k); dst (rr, k, j, q)
                    src = bass.AP(tensor=psf.tensor, offset=psf[:, 0].offset,
                                  ap=[list(psf[:, :].ap[0]), [1, KK], [KK, njj]])
                    dst = bass.AP(tensor=varg.tensor,
                                  offset=varg[0, 0, 0, q].offset,
                                  ap=[[varg[:, 0, 0, 0].ap[0][0], 32],
                                      [njj * 8, KK], [8, njj]])
                    nc.scalar.copy(dst, src)

                idxg_q = wq2.tile([32, KK, njj, 8], I16, tag="idxg")
                nc.vector.tensor_tensor(out=idxg_q[:, :, :, :],
                                        in0=varg[:, :, :, :],
                                        in1=baseg[0:32, Q, :, j0l:j0l + njj, :],
                                        op=mm.add)
                return v4_q, idxg_q

            def transpose_pairs(sampt, pso_p, ts, nblk):
                """Transpose tap-pairs ts (ready once tap 2t+1 is computed) for
                all jl into [128, 512] f16 rhs tiles, then immediately fold
                them into the per-block PSUM accumulation (start on t==0) so
                only the single-tap work remains after the last gather."""
                for t in ts:
                    for blk in range(nblk):
                        rhs_t = rpool.tile([128, 512], F16, tag="rhs")
                        for i in range(4):
                            jl = 4 * blk + i
                            pstp = psTp.tile([128, 128], F16, tag="pstp")
                            nc.tensor.transpose(
                                pstp, sampt[:, jl, 2 * t:2 * t + 2, :], idf16)
                            nc.scalar.copy(rhs_t[:, i * 128:(i + 1) * 128],
                                           pstp)
                        if t == 0:
                            pso_t = psO.tile([O, 512], F32, tag="pso")
                            pso_p[blk] = pso_t
                        nc.tensor.matmul(pso_p[blk], w2p[:, t, :], rhs_t,
                                         start=(t == 0), stop=False)

            def finish_quarter(sampt, pso_p, jj0, nblk):
                """Single-tap transposes + final accumulating matmul + output."""
                for blk in range(nblk):
                    rhs_s = rspool.tile([C, 512], F16, tag="rhss")
                    for i in range(4):
                        jl = 4 * blk + i
                        psts = psTp.tile([C, 128], F16, tag="pstp")
                        nc.tensor.transpose(psts, sampt[:, jl, 8, :], idf16)
                        nc.scalar.copy(rhs_s[:, i * 128:(i + 1) * 128], psts)
                    nc.tensor.matmul(pso_p[blk], w2s, rhs_s,
                                     start=False, stop=True)
                    oq = opool.tile([O, 512], F32)
                    nc.scalar.copy(oq, pso_p[blk])
                    nc.sync.dma_start(out=OUT[:, jj0 * 128 + blk * 512:
                                              jj0 * 128 + (blk + 1) * 512],
                                      in_=oq)

            # ---------- software-pipelined main loop ----------
            # prep is issued one slab ahead so its DVE/PE ops never queue
            # behind the current slab's tap work in the strict-FIFO engine
            # queues; gathers for the next slab start with zero boundary
            # bubble. The first quarter runs as two 8-jj slabs so the first
            # gather issues after ~half the serial prep latency.
            slabs = [(0, 4), (4, 4), (8, 8), (16, 16), (32, 16),
                     (48, 8), (56, 8)]
            vi = prep_quarter(*slabs[0])
            for si, (jj0, njj) in enumerate(slabs):
                vi_next = (prep_quarter(*slabs[si + 1])
                           if si + 1 < len(slabs) else None)
                v4_q, idxg_q = vi
                nblk = njj // 4
                sampt = spool.tile([128, JQ, KK, C], F16)
                pso_p = [None] * nblk
                for k in range(KK):
                    g = gpool.tile([128, JQ, 256], F16)
                    nc.gpsimd.dma_gather(
                        out_ap=g[:, 0:njj, :],
                        in_ap=TBL[:, :],
                        idxs_ap=idxg_q[:, k, :, :],
                        num_idxs=njj * 128,
                        num_idxs_reg=njj * 128,
                        elem_size=256, single_packet=False,
                    )
                    v4s = v4_q[:, :, k, :]
                    v4v = _bcast(v4s, 2, C)
                    h = hpool.tile([128, JQ, C, 4], F16, tag="h")
                    nc.vector.tensor_tensor(
                        out=h[:, 0:njj, :, :],
                        in0=g[:, 0:njj, :].rearrange("p j (c f) -> p j c f", c=C),
                        in1=v4v, op=mm.mult)
                    # 4-corner sum as two adds: first runs in DVE 2x mode
                    # (packed fp16 pairs), beats tensor_reduce's 1x.
                    s2 = hpool.tile([128, JQ, C, 2], F16, tag="s2")
                    with nc.allow_low_precision(reason="4-corner fp16 sum"):
                        nc.vector.tensor_tensor(
                            out=s2[:, 0:njj, :, :], in0=h[:, 0:njj, :, 0:2],
                            in1=h[:, 0:njj, :, 2:4], op=mm.add)
                        nc.vector.tensor_tensor(
                            out=sampt[:, 0:njj, k, :], in0=s2[:, 0:njj, :, 0],
                            in1=s2[:, 0:njj, :, 1], op=mm.add)
                    if k == 3:
                        transpose_pairs(sampt, pso_p, (0, 1), nblk)
                    if k == 7:
                        transpose_pairs(sampt, pso_p, (2, 3), nblk)
                finish_quarter(sampt, pso_p, jj0, nblk)
                vi = vi_next
    nc.compile()
    return nc


# ======================= runner =======================
_NC = None


def _get_nc():
    global _NC
    if _NC is None:
        _NC = build()
    return _NC


def kernel(x, offset_weight, offset_bias, weight):
    from concourse.bass_utils import run_bass_kernel_spmd
    x = np.asarray(x)
    offset_weight = np.asarray(offset_weight)
    offset_bias = np.asarray(offset_bias)
    weight = np.asarray(weight)
    in_maps = host_prep(x, offset_weight, offset_bias, weight)
    nc = _get_nc()
    res = run_bass_kernel_spmd(nc, in_maps, core_ids=list(range(8)))
    return host_post([r["OUT"] for r in res.results])


# revision 57
# speedup vs baseline: 1.0086x; 1.0086x over previous
"""Deformable Conv2d Trainium kernel: host prep + bass program builder.

Sharding: 8 cores = 4 batches x 2 height-halves; each core computes
out[b, :, h0:h0+32, :] (128 out-ch x 8192 positions).

Position layout per core: pos = jj*128 + p, jj in [0,64), p in [0,128);
ho_local = jj//2, wo = (jj%2)*128 + p.

Device pipeline (per-quarter software pipeline; the Q7 dma_gather
descriptor-generation stream [~16.4us per 2048-idx gather] is the
bottleneck, so all prep (B-E) for quarter Q+1 and all consume work
(DVE weighting, PE transposes/matmuls, output DMA) for quarter Q-1
must hide underneath quarter Q's 9 gathers):
  per quarter Q (2048 positions = 16 jj blocks):
    B: offset conv om_q[27, 2048] (PE, fp16 inputs, fp32 accum)
    C: omT via PE transposes -> omt_q [128, 16, 27] f32
    D: bilinear weight math (DVE/ACT) on [128, 144] slices
       -> V4 [128, 64, 9, 4] fp16, varf_q [128,144] f32
    E: idx fold via 8 permutation matmuls -> IDXG [128, 4, 9, 16, 8] int16
    F: per tap k: dma_gather (DRAM table, 512B items) -> G [128,16,256] fp16;
       H = G*V4 (DVE); corner-reduce -> SAMPT;
       PE transposes (batched 4-jl into [128,512] f16 PSUM) -> RHS [128,512];
       weight-stationary matmuls (5 ldw per 512-pos block) -> PSUM -> OUT
"""
import numpy as np

import concourse.bass as bass
import concourse.mybir as mybir
import concourse.tile as tile
from concourse import bacc

F32 = mybir.dt.float32
F16 = mybir.dt.float16
I16 = mybir.dt.int16
I32 = mybir.dt.int32

C = 64
O = 128
H = 64
W = 256
KK = 9
BD = 4                 # vertical halo margin (max |dy| must be < BD)
MX = 5                 # horizontal margin (max |dx| must be < MX)
RT = 42                # table rows: covers y0 in [h0-1-BD, h0+32+BD] inclusive
PITCH = 384            # table row pitch (multiple of 128, >= 270)
TCW = 268              # valid table cols: tc = x0 + MX + 1 in [0, 267]
XCW = TCW + 1          # padded x-slice cols (item needs tc+1)
NIT = RT * PITCH       # table items (16128)
NPOS = 32 * W          # 8192 positions per core
NJJ = 64               # pos blocks of 128
NQ = 4                 # quarters
JQ = 16                # jj per quarter
NF = NJJ * KK          # 576
QF = JQ * KK           # 144 free elems per quarter in (jj,k) layout


def _xpad_slice(xb, h0):
    """[C, RT, XCW] zero-padded fp16 slice; rows y_base..y_base+RT-1, col tc=xg+MX+1."""
    y_base = h0 - 1 - BD
    xp = np.zeros((C, RT, XCW), np.float16)
    r0 = max(0, -y_base)
    r1 = min(RT, H - y_base)
    xp[:, r0:r1, MX + 1:MX + 1 + W] = xb[:, y_base + r0:y_base + r1, :].astype(np.float16)
    return xp


def _xp2(xp):
    """[128, RT, XCW]: partitions 0:64 = xp, 64:128 = xp shifted one row up
    (row r holds xp row r+1), so a 128-partition contraction covers vertical
    tap pairs (ki=0,1) in one matmul pass."""
    sh = np.zeros_like(xp)
    sh[:, :RT - 1] = xp[:, 1:]
    return np.concatenate([xp, sh], axis=0)


def _table(xp):
    """[NIT, 256] fp16 gather table; item (r, tc) = per-c [v00, v01, v10, v11]."""
    it = np.zeros((RT, PITCH, C, 4), np.float16)
    it[:RT - 1, :TCW, :, 0] = xp[:, :RT - 1, :TCW].transpose(1, 2, 0)
    it[:RT - 1, :TCW, :, 1] = xp[:, :RT - 1, 1:TCW + 1].transpose(1, 2, 0)
    it[:RT - 1, :TCW, :, 2] = xp[:, 1:RT, :TCW].transpose(1, 2, 0)
    it[:RT - 1, :TCW, :, 3] = xp[:, 1:RT, 1:TCW + 1].transpose(1, 2, 0)
    return it.reshape(NIT, 256)


def host_prep(x, offset_weight, offset_bias, weight):
    """Returns list of 8 in_map dicts; core order = (b, hh)."""
    jj = np.arange(NJJ)
    kv = np.arange(KK)
    ki = kv // 3
    kj = kv % 3
    p = np.arange(128)

    # vertical-pair offset-conv weights: row u*64+c of pass kj = w[o, c, ki=u, kj]
    owr = offset_weight.reshape(27, C, 3, 3)
    ow2p2 = np.zeros((128, 3, 27), np.float16)
    for u in range(2):
        for q in range(3):
            ow2p2[u * 64:(u + 1) * 64, q, :] = owr[:, :, u, q].T
    ow2s2 = np.zeros((C, 3, 27), np.float16)
    for q in range(3):
        ow2s2[:, q, :] = owr[:, :, 2, q].T
    ob = offset_bias.reshape(27, 1).astype(np.float32)
    w2 = weight.reshape(O, C, KK)
    w2p = np.zeros((128, 4, 128), np.float16)
    for t in range(4):
        w2p[:64, t, :] = w2[:, :, 2 * t].T.astype(np.float16)
        w2p[64:, t, :] = w2[:, :, 2 * t + 1].T.astype(np.float16)
    w2s = np.ascontiguousarray(w2[:, :, 8].T.astype(np.float16))          # [64, 128]

    x0b = ((jj[None, :, None] % 2) * 128 + p[:, None, None] + kj[None, None, :] - 1
           ).astype(np.float32).reshape(128, NF)
    base = ((jj[None, :, None] // 2 + BD + ki[None, None, :]) * PITCH
            + (jj[None, :, None] % 2) * 128 + p[:, None, None]
            + kj[None, None, :] + MX).astype(np.float32)                   # [128, 64, 9]
    # BASEG [128, NQ, KK, JQ, 8] f32: [rr, Q, k, j, q] = base[q*16+rr, Q*16+j, k]
    baseg = np.zeros((16, NQ, KK, JQ, 8), np.float32)
    for q in range(8):
        for rr in range(16):
            b16 = base[q * 16 + rr]                      # [64, 9]
            baseg[rr, :, :, :, q] = b16.reshape(NQ, JQ, KK).transpose(0, 2, 1)
    baseg = np.tile(baseg, (8, 1, 1, 1, 1)).reshape(128, NQ * KK * JQ * 8)

    # E-phase fold matrices: matmul q maps varf partitions q*16..q*16+15 to a
    # [32, nf] PSUM tile with rows 16-31 duplicating rows 0-15, so the idx add
    # writes partitions 0-31 directly (the gather ucode reads exactly 32) and
    # no broadcast DMA is needed.
    permq = np.zeros((128, 8, 32), dtype=np.float32)
    for q in range(8):
        for rr in range(16):
            permq[q * 16 + rr, q, rr] = 1.0
            permq[q * 16 + rr, q, 16 + rr] = 1.0
    permq = permq.reshape(128, 256)
    idf16 = np.eye(128, dtype=np.float16)
    idf32 = np.eye(27, dtype=np.float32)
    # per-partition constants for 2-input DVE ops (1-input tensor_scalar/copy
    # ops run in DVE 2-port mode, which contends with the Q7 gather
    # descriptor-gen for SBUF ports and stalls up to 12us mid-stream):
    # cols: 0, 1, -1, H-1, H-2, W-1, W-2, round-magic (1.5*2^23)
    cb = np.tile(np.array([0.0, 1.0, -1.0, H - 1, H - 2, W - 1, W - 2,
                           12582912.0], np.float32), (128, 1))

    in_maps = []
    for core in range(8):
        b, hh = core // 2, core % 2
        h0 = hh * 32
        xp = _xpad_slice(x[b], h0)
        hoky = ((h0 + jj[None, :, None] // 2 + ki[None, None, :] - 1)
                * np.ones((128, 1, 1))).astype(np.float32)
        in_maps.append({
            "XP2": np.ascontiguousarray(_xp2(xp).reshape(128, RT * XCW)),
            "TBL": np.ascontiguousarray(_table(xp)),
            "OW2P2": np.ascontiguousarray(ow2p2.reshape(128, 3 * 27)),
            "OW2S2": np.ascontiguousarray(ow2s2.reshape(C, 3 * 27)),
            "OB": ob,
            "W2P": w2p, "W2S": w2s,
            "HOKY": np.ascontiguousarray(hoky.reshape(128, NF)), "X0B": x0b,
            "BASEG": baseg, "PERMQ": permq, "IDF16": idf16, "IDF32": idf32,
            "CB": cb,
        })
    return in_maps


def host_post(outs):
    """outs: list of 8 [128, 8192] f32 -> [4, 128, 64, 256]."""
    y = np.zeros((4, O, H, W), np.float32)
    for core, o in enumerate(outs):
        b, hh = core // 2, core % 2
        v = np.asarray(o).reshape(O, 32, 2, 128).reshape(O, 32, 256)
        y[b, :, hh * 32:hh * 32 + 32, :] = v
    return y


def _bcast(ap, dim, n):
    """Insert a [0, n] broadcast dim at free position `dim` (1-based in ap list)."""
    newap = [list(d) for d in ap.ap]
    newap.insert(dim, [0, n])
    return bass.AP(tensor=ap.tensor, offset=ap.offset, ap=newap)


def build(nc=None):
    if nc is None:
        nc = bacc.Bacc("TRN2", target_bir_lowering=False, debug=False)
    XP2 = nc.dram_tensor("XP2", [128, RT * XCW], F16, kind="ExternalInput")
    TBL = nc.dram_tensor("TBL", [NIT, 256], F16, kind="ExternalInput")
    OW2P2 = nc.dram_tensor("OW2P2", [128, 3 * 27], F16, kind="ExternalInput")
    OW2S2 = nc.dram_tensor("OW2S2", [C, 3 * 27], F16, kind="ExternalInput")
    OB = nc.dram_tensor("OB", [27, 1], F32, kind="ExternalInput")
    W2P = nc.dram_tensor("W2P", [128, 4, 128], F16, kind="ExternalInput")
    W2S = nc.dram_tensor("W2S", [C, 128], F16, kind="ExternalInput")
    HOKY = nc.dram_tensor("HOKY", [128, NF], F32, kind="ExternalInput")
    X0B = nc.dram_tensor("X0B", [128, NF], F32, kind="ExternalInput")
    BASEG = nc.dram_tensor("BASEG", [128, NQ * KK * JQ * 8], F32, kind="ExternalInput")
    PERMQ = nc.dram_tensor("PERMQ", [128, 8 * 32], F32, kind="ExternalInput")
    IDF16 = nc.dram_tensor("IDF16", [128, 128], F16, kind="ExternalInput")
    IDF32 = nc.dram_tensor("IDF32", [27, 27], F32, kind="ExternalInput")
    CB = nc.dram_tensor("CB", [128, 8], F32, kind="ExternalInput")
    OUT = nc.dram_tensor("OUT", [O, NPOS], F32, kind="ExternalOutput")

    mm = mybir.AluOpType

    with tile.TileContext(nc) as tc:
        with (
            tc.tile_pool(name="const", bufs=1) as cpool,
            tc.tile_pool(name="wq", bufs=1) as wq,
            tc.tile_pool(name="wq2", bufs=2) as wq2,
            tc.tile_pool(name="gat", bufs=3) as gpool,
            tc.tile_pool(name="hb", bufs=2) as hpool,
            tc.tile_pool(name="samp", bufs=2) as spool,
            tc.tile_pool(name="rhs", bufs=12) as rpool,
            tc.tile_pool(name="rhss", bufs=5) as rspool,
            tc.tile_pool(name="oq", bufs=2) as opool,
            tc.tile_pool(name="psB", bufs=1, space="PSUM") as psB,
            tc.tile_pool(name="psP", bufs=1, space="PSUM") as psP,
            tc.tile_pool(name="psTp", bufs=2, space="PSUM") as psTp,
            tc.tile_pool(name="psO", bufs=4, space="PSUM") as psO,
        ):
            # ---------- constants, ordered by when the serial prep(0) chain
            # needs them: tiny offset-conv weights, then just enough xp2 rows
            # for the first slabs' conv; the bulky baseg/xp2-tail/w2p queue
            # after everything that gates the first gather ----------
            ow2p2 = cpool.tile([128, 3, 27], F16)
            nc.sync.dma_start(out=ow2p2,
                              in_=OW2P2[:, :].rearrange("c (k o) -> c k o", k=3))
            ow2s2 = cpool.tile([C, 3, 27], F16)
            nc.sync.dma_start(out=ow2s2,
                              in_=OW2S2[:, :].rearrange("c (k o) -> c k o", k=3))
            ob = cpool.tile([27, 1], F32)
            nc.sync.dma_start(out=ob, in_=OB[:, :])
            idf32 = cpool.tile([27, 27], F32)
            nc.sync.dma_start(out=idf32, in_=IDF32[:, :])
            cb = cpool.tile([128, 8], F32)
            nc.sync.dma_start(out=cb, in_=CB[:, :])
            xp2 = cpool.tile([128, RT, XCW], F16)
            nc.sync.dma_start(out=xp2[:, 0:16, :],
                              in_=XP2[:, 0:16 * XCW].rearrange(
                                  "c (r w) -> c r w", r=16))

            def cbv(col, n, p=128):
                """[p, n] broadcast AP of constant column `col` of cb."""
                a = cb[0:p, col:col + 1]
                return bass.AP(tensor=a.tensor, offset=a.offset,
                               ap=[list(a.ap[0]), [0, n]])
            hoky = cpool.tile([128, NF], F32)
            nc.sync.dma_start(out=hoky, in_=HOKY[:, :])
            x0b = cpool.tile([128, NF], F32)
            nc.sync.dma_start(out=x0b, in_=X0B[:, :])
            permq = cpool.tile([128, 8, 32], F32)
            nc.sync.dma_start(out=permq,
                              in_=PERMQ[:, :].rearrange("c (q m) -> c q m", q=8))
            baseg = cpool.tile([128, NQ, KK, JQ, 8], F32)
            nc.sync.dma_start(out=baseg,
                              in_=BASEG[:, :].rearrange(
                                  "r (a k j q) -> r a k j q", a=NQ, k=KK, j=JQ))
            nc.sync.dma_start(out=xp2[:, 16:RT, :],
                              in_=XP2[:, 16 * XCW:].rearrange(
                                  "c (r w) -> c r w", r=RT - 16))
            w2p = cpool.tile([128, 4, 128], F16)
            nc.sync.dma_start(out=w2p, in_=W2P[:, :, :])
            w2s = cpool.tile([C, 128], F16)
            nc.sync.dma_start(out=w2s, in_=W2S[:, :])
            idf16 = cpool.tile([128, 128], F16)
            nc.sync.dma_start(out=idf16, in_=IDF16[:, :])

            def prep_quarter(jj0, njj):
                """Phases B-E for a slab of njj jj-blocks starting at jj0.

                Returns (v4_q, idxg_q) double-buffered per-slab tiles so the
                next slab's prep never WAR-stalls on this slab's gathers.
                The first quarter runs as two 8-jj slabs to halve the serial
                prep latency ahead of the first gather."""
                nf = njj * KK
                s = slice(jj0 * KK, (jj0 + njj) * KK)   # slice in (jj,k) layout
                Q = jj0 // JQ
                j0l = jj0 % JQ

                # ---- B: offset conv (vertical tap pairs; 6 passes) ----
                om_q = wq.tile([27, njj * 128], F32, tag="om")
                for cc in range(njj // 4):
                    ch = jj0 // 4 + cc
                    ps = psB.tile([27, 512], F32, tag="psom")
                    r = 2 * ch + BD
                    for kj in range(3):
                        rhs = xp2[:, r:r + 2, kj + MX:kj + MX + W]
                        nc.tensor.matmul(ps, ow2p2[:, kj, :], rhs,
                                         start=(kj == 0), stop=False)
                    for kj in range(3):
                        rhs = xp2[0:C, r + 2:r + 4, kj + MX:kj + MX + W]
                        nc.tensor.matmul(ps, ow2s2[:, kj, :], rhs,
                                         start=False, stop=(kj == 2))
                    nc.vector.scalar_tensor_tensor(
                        out=om_q[:, cc * 512:(cc + 1) * 512], in0=ps,
                        scalar=ob, in1=cbv(0, 512, p=27),
                        op0=mm.add, op1=mm.add)

                # ---- C: transpose om -> omt_q [128, 16, 27] ----
                omt_q = wq.tile([128, njj, 27], F32, tag="omt")
                for j in range(njj):
                    pst = psP.tile([128, 27], F32, tag="prep_ps")
                    nc.tensor.transpose(pst, om_q[:, j * 128:(j + 1) * 128], idf32)
                    nc.scalar.copy(omt_q[:, j, :], pst)

                # ---- D: weight math on [128, 144] ----
                DY = omt_q[:, :, 0:18:2]
                DX = omt_q[:, :, 1:18:2]
                MZ = omt_q[:, :, 18:27]

                msig = wq.tile([128, nf], F32, tag="msig")
                nc.scalar.activation(out=msig, in_=MZ,
                                     func=mybir.ActivationFunctionType.Sigmoid)

                def floor_frac(src_ap, ftag):
                    # floor via fp32 magic rounding (t = (src+1.5*2^23)-1.5*2^23
                    # rounds to integer; then f = t - (src < t)). All 2-input
                    # TT ops: no DVE 2-port mode, no Q7 port contention.
                    tm = wq.tile([128, nf], F32, tag="flr_m")
                    nc.vector.tensor_tensor(out=tm, in0=src_ap,
                                            in1=cbv(7, nf), op=mm.add)
                    tf = wq.tile([128, nf], F32, tag="flr_f")
                    nc.vector.tensor_tensor(out=tf, in0=tm,
                                            in1=cbv(7, nf), op=mm.subtract)
                    lt = wq.tile([128, nf], F32, tag="flr_lt")
                    nc.vector.tensor_tensor(out=lt, in0=src_ap, in1=tf,
                                            op=mm.is_lt)
                    fl = wq.tile([128, nf], F32, tag=ftag + "_f")
                    nc.vector.tensor_tensor(out=fl, in0=tf, in1=lt,
                                            op=mm.subtract)
                    fr = wq.tile([128, nf], F32, tag=ftag + "_l")
                    nc.vector.tensor_tensor(out=fr, in0=src_ap, in1=fl,
                                            op=mm.subtract)
                    return fl, fr

                fy, ly = floor_frac(DY, "fy")
                fx, lx = floor_frac(DX, "fx")

                y0 = wq.tile([128, nf], F32, tag="y0")
                nc.vector.tensor_tensor(out=y0, in0=fy, in1=hoky[:, s], op=mm.add)
                x0 = wq.tile([128, nf], F32, tag="x0")
                nc.vector.tensor_tensor(out=x0, in0=fx, in1=x0b[:, s], op=mm.add)

                def in_range(src, lo_col, hi_col, out_tag):
                    a = wq.tile([128, nf], F32, tag="rng_a")
                    nc.vector.tensor_tensor(out=a, in0=src,
                                            in1=cbv(lo_col, nf), op=mm.is_ge)
                    bq = wq.tile([128, nf], F32, tag="rng_b")
                    nc.vector.tensor_tensor(out=bq, in0=src,
                                            in1=cbv(hi_col, nf), op=mm.is_le)
                    rr = wq.tile([128, nf], F32, tag=out_tag)
                    nc.vector.tensor_tensor(out=rr, in0=a, in1=bq, op=mm.mult)
                    return rr

                vy0 = in_range(y0, 0, 3, "vy0")
                vy1 = in_range(y0, 2, 4, "vy1")
                vx0 = in_range(x0, 0, 5, "vx0")
                vx1 = in_range(x0, 2, 6, "vx1")

                ily = wq.tile([128, nf], F32, tag="ily")
                nc.vector.scalar_tensor_tensor(out=ily, in0=ly, scalar=-1.0,
                                               in1=cbv(1, nf),
                                               op0=mm.mult, op1=mm.add)
                ilx = wq.tile([128, nf], F32, tag="ilx")
                nc.vector.scalar_tensor_tensor(out=ilx, in0=lx, scalar=-1.0,
                                               in1=cbv(1, nf),
                                               op0=mm.mult, op1=mm.add)

                a0 = wq.tile([128, nf], F32, tag="a0")
                nc.vector.tensor_tensor(out=a0, in0=msig, in1=ily, op=mm.mult)
                nc.vector.tensor_tensor(out=a0, in0=a0, in1=vy0, op=mm.mult)
                a1 = wq.tile([128, nf], F32, tag="a1")
                nc.vector.tensor_tensor(out=a1, in0=msig, in1=ly, op=mm.mult)
                nc.vector.tensor_tensor(out=a1, in0=a1, in1=vy1, op=mm.mult)
                b0 = wq.tile([128, nf], F32, tag="b0")
                nc.vector.tensor_tensor(out=b0, in0=ilx, in1=vx0, op=mm.mult)
                b1 = wq.tile([128, nf], F32, tag="b1")
                nc.vector.tensor_tensor(out=b1, in0=lx, in1=vx1, op=mm.mult)

                v4_q = wq2.tile([128, njj, KK, 4], F16, tag="v4")
                nc.vector.tensor_tensor(out=v4_q[:, :, :, 0], in0=a0, in1=b0, op=mm.mult)
                nc.vector.tensor_tensor(out=v4_q[:, :, :, 1], in0=a0, in1=b1, op=mm.mult)
                nc.vector.tensor_tensor(out=v4_q[:, :, :, 2], in0=a1, in1=b0, op=mm.mult)
                nc.vector.tensor_tensor(out=v4_q[:, :, :, 3], in0=a1, in1=b1, op=mm.mult)

                varf = wq.tile([128, nf], F32, tag="varf")
                nc.vector.scalar_tensor_tensor(out=varf, in0=fy, scalar=float(PITCH),
                                               in1=fx, op0=mm.mult, op1=mm.add)

                # ---- E: idx fold (8 perm matmuls -> varg [32,(k,j,q)]).
                # permq duplicates rows into partitions 16-31, covering both
                # Q7 core idx reads with no broadcast DMA. ----
                varg = wq.tile([32, KK, njj, 8], F32, tag="varg")
                for q in range(8):
                    psf = psP.tile([32, nf], F32, tag="prep_ps")
                    nc.tensor.matmul(psf, permq[:, q, :],
                                     varf[:, :], start=True, stop=True)
                    # psf free = (j, k); dst (rr, k, j, q)
                    src = bass.AP(tensor=psf.tensor, offset=psf[:, 0].offset,
                                  ap=[list(psf[:, :].ap[0]), [1, KK], [KK, njj]])
                    dst = bass.AP(tensor=varg.tensor,
                                  offset=varg[0, 0, 0, q].offset,
                                  ap=[[varg[:, 0, 0, 0].ap[0][0], 32],
                                      [njj * 8, KK], [8, njj]])
                    nc.scalar.copy(dst, src)

                idxg_q = wq2.tile([32, KK, njj, 8], I16, tag="idxg")
                nc.vector.tensor_tensor(out=idxg_q[:, :, :, :],
                                        in0=varg[:, :, :, :],
                                        in1=baseg[0:32, Q, :, j0l:j0l + njj, :],
                                        op=mm.add)
                return v4_q, idxg_q

            def transpose_pairs(sampt, pso_p, ts, nblk):
                """Transpose tap-pairs ts (ready once tap 2t+1 is computed) for
                all jl into [128, 512] f16 rhs tiles, then immediately fold
                them into the per-block PSUM accumulation (start on t==0) so
                only the single-tap work remains after the last gather."""
                for t in ts:
                    for blk in range(nblk):
                        rhs_t = rpool.tile([128, 512], F16, tag="rhs")
                        for i in range(4):
                            jl = 4 * blk + i
                            pstp = psTp.tile([128, 128], F16, tag="pstp")
                            nc.tensor.transpose(
                                pstp, sampt[:, jl, 2 * t:2 * t + 2, :], idf16)
                            nc.scalar.copy(rhs_t[:, i * 128:(i + 1) * 128],
                                           pstp)
                        if t == 0:
                            pso_t = psO.tile([O, 512], F32, tag="pso")
                            pso_p[blk] = pso_t
                        nc.tensor.matmul(pso_p[blk], w2p[:, t, :], rhs_t,
                                         start=(t == 0), stop=False)

            def finish_quarter(sampt, pso_p, jj0, nblk):
                """Single-tap transposes + final accumulating matmul + output."""
                for blk in range(nblk):
                    rhs_s = rspool.tile([C, 512], F16, tag="rhss")
                    for i in range(4):
                        jl = 4 * blk + i
                        psts = psTp.tile([C, 128], F16, tag="pstp")
                        nc.tensor.transpose(psts, sampt[:, jl, 8, :], idf16)
                        nc.scalar.copy(rhs_s[:, i * 128:(i + 1) * 128], psts)
                    nc.tensor.matmul(pso_p[blk], w2s, rhs_s,
                                     start=False, stop=True)
                    oq = opool.tile([O, 512], F32)
                    nc.scalar.copy(oq, pso_p[blk])
                    nc.sync.dma_start(out=OUT[:, jj0 * 128 + blk * 512:
                                              jj0 * 128 + (blk + 1) * 512],
                                      in_=oq)

            # ---------- software-pipelined main loop ----------
            # prep is issued one slab ahead so its DVE/PE ops never queue
            # behind the current slab's tap work in the strict-FIFO engine
            # queues; gathers for the next slab start with zero boundary
            # bubble. The first quarter runs as two 8-jj slabs so the first
            # gather issues after ~half the serial prep latency.
            slabs = [(0, 4), (4, 4), (8, 8), (16, 16), (32, 16),
                     (48, 8), (56, 8)]
            vi = prep_quarter(*slabs[0])
            for si, (jj0, njj) in enumerate(slabs):
                vi_next = (prep_quarter(*slabs[si + 1])
                           if si + 1 < len(slabs) else None)
                v4_q, idxg_q = vi
                nblk = njj // 4
                sampt = spool.tile([128, JQ, KK, C], F16)
                pso_p = [None] * nblk
                for k in range(KK):
                    g = gpool.tile([128, JQ, 256], F16)
                    nc.gpsimd.dma_gather(
                        out_ap=g[:, 0:njj, :],
                        in_ap=TBL[:, :],
                        idxs_ap=idxg_q[:, k, :, :],
                        num_idxs=njj * 128,
                        num_idxs_reg=njj * 128,
                        elem_size=256, single_packet=False,
                    )
                    v4s = v4_q[:, :, k, :]
                    v4v = _bcast(v4s, 2, C)
                    h = hpool.tile([128, JQ, C, 4], F16, tag="h")
                    nc.vector.tensor_tensor(
                        out=h[:, 0:njj, :, :],
                        in0=g[:, 0:njj, :].rearrange("p j (c f) -> p j c f", c=C),
                        in1=v4v, op=mm.mult)
                    # 4-corner sum as two adds: first runs in DVE 2x mode
                    # (packed fp16 pairs), beats tensor_reduce's 1x.
                    s2 = hpool.tile([128, JQ, C, 2], F16, tag="s2")
                    with nc.allow_low_precision(reason="4-corner fp16 sum"):
                        nc.vector.tensor_tensor(
                            out=s2[:, 0:njj, :, :], in0=h[:, 0:njj, :, 0:2],
                            in1=h[:, 0:njj, :, 2:4], op=mm.add)
                        nc.vector.tensor_tensor(
                            out=sampt[:, 0:njj, k, :], in0=s2[:, 0:njj, :, 0],
                            in1=s2[:, 0:njj, :, 1], op=mm.add)
                    if k == 3:
                        transpose_pairs(sampt, pso_p, (0, 1), nblk)
                    if k == 7:
                        transpose_pairs(sampt, pso_p, (2, 3), nblk)
                finish_quarter(sampt, pso_p, jj0, nblk)
                vi = vi_next
    nc.compile()
    return nc


# ======================= runner =======================
_NC = None


def _get_nc():
    global _NC
    if _NC is None:
        _NC = build()
    return _NC


def kernel(x, offset_weight, offset_bias, weight):
    from concourse.bass_utils import run_bass_kernel_spmd
    x = np.asarray(x)
    offset_weight = np.asarray(offset_weight)
    offset_bias = np.asarray(offset_bias)
    weight = np.asarray(weight)
    in_maps = host_prep(x, offset_weight, offset_bias, weight)
    nc = _get_nc()
    res = run_bass_kernel_spmd(nc, in_maps, core_ids=list(range(8)))
    return host_post([r["OUT"] for r in res.results])


# revision 58
# speedup vs baseline: 1.0111x; 1.0025x over previous
"""Deformable Conv2d Trainium kernel: host prep + bass program builder.

Sharding: 8 cores = 4 batches x 2 height-halves; each core computes
out[b, :, h0:h0+32, :] (128 out-ch x 8192 positions).

Position layout per core: pos = jj*128 + p, jj in [0,64), p in [0,128);
ho_local = jj//2, wo = (jj%2)*128 + p.

Device pipeline: a slab-wise software pipeline over the Q7 dma_gather
descriptor-generation stream, which is the hard bottleneck (~0.45us
fixed + 7.9ns/idx per gather, one descriptor per bilinear sample,
single SWDGE queue). Slabs of jj-blocks -- (0,4),(4,4),(8,8),(16,16),
(32,16),(48,8),(56,8) -- small at the head so the first gather issues
after ~1/4 of the serial prep latency, small at the tail so little
work remains after the last gather. Prep for slab i+1 is issued ahead
of slab i's taps; all DVE/PE/ACT/DMA consume work hides under the
gather stream. Per slab (njj jj-blocks):
  B: offset conv om_q[27, njj*128] (PE, fp16 in, fp32 accum; vertical
     tap pairs via the row-shifted XP2 partition layout: 6 passes not 9)
  C: omT via PE transposes -> omt_q [128, njj, 27] f32
  D: bilinear weight math on [128, njj*9] slices -> V4 [128,njj,9,4] f16.
     All ops are 2-input DVE forms (TT vs broadcast constants, stt,
     fp32 magic-rounding floor) -- 1-input ops run in DVE 2-port mode
     and contend with Q7 descriptor-gen for SBUF ports (12us stalls).
  E: idx fold via 8 permutation matmuls whose [128, 32] fold matrices
     duplicate rows into partitions 16-31 (both Q7 cores' idx reads,
     no broadcast DMA) -> idxg_q [32, 9, njj, 8] int16
  F: per tap k: dma_gather (DRAM table, 512B items = [c, 4 corners]
     f16) -> G [128, njj, 256]; H = G*V4 (DVE 2x); y-pair add -> SAMPT;
     at k==3/k==7 tap-pair PE transposes (4-jl batches -> [128,512] f16
     rhs) feed progressive PSUM accumulation (psO bank per 512-pos
     block held open); after k==8 only single-tap transposes + the
     stop matmul + output DMA remain.
"""
import numpy as np

import concourse.bass as bass
import concourse.mybir as mybir
import concourse.tile as tile
from concourse import bacc

F32 = mybir.dt.float32
F16 = mybir.dt.float16
I16 = mybir.dt.int16
I32 = mybir.dt.int32

C = 64
O = 128
H = 64
W = 256
KK = 9
BD = 4                 # vertical halo margin (max |dy| must be < BD)
MX = 5                 # horizontal margin (max |dx| must be < MX)
RT = 42                # table rows: covers y0 in [h0-1-BD, h0+32+BD] inclusive
PITCH = 384            # table row pitch (multiple of 128, >= 270)
TCW = 268              # valid table cols: tc = x0 + MX + 1 in [0, 267]
XCW = TCW + 1          # padded x-slice cols (item needs tc+1)
NIT = RT * PITCH       # table items (16128)
NPOS = 32 * W          # 8192 positions per core
NJJ = 64               # pos blocks of 128
NQ = 4                 # quarters
JQ = 16                # jj per quarter
NF = NJJ * KK          # 576
QF = JQ * KK           # 144 free elems per quarter in (jj,k) layout


def _xpad_slice(xb, h0):
    """[C, RT, XCW] zero-padded fp16 slice; rows y_base..y_base+RT-1, col tc=xg+MX+1."""
    y_base = h0 - 1 - BD
    xp = np.zeros((C, RT, XCW), np.float16)
    r0 = max(0, -y_base)
    r1 = min(RT, H - y_base)
    xp[:, r0:r1, MX + 1:MX + 1 + W] = xb[:, y_base + r0:y_base + r1, :].astype(np.float16)
    return xp


def _xp2(xp):
    """[128, RT, XCW]: partitions 0:64 = xp, 64:128 = xp shifted one row up
    (row r holds xp row r+1), so a 128-partition contraction covers vertical
    tap pairs (ki=0,1) in one matmul pass."""
    sh = np.zeros_like(xp)
    sh[:, :RT - 1] = xp[:, 1:]
    return np.concatenate([xp, sh], axis=0)


def _table(xp):
    """[NIT, 256] fp16 gather table; item (r, tc) = per-c [v00, v01, v10, v11]."""
    it = np.zeros((RT, PITCH, C, 4), np.float16)
    it[:RT - 1, :TCW, :, 0] = xp[:, :RT - 1, :TCW].transpose(1, 2, 0)
    it[:RT - 1, :TCW, :, 1] = xp[:, :RT - 1, 1:TCW + 1].transpose(1, 2, 0)
    it[:RT - 1, :TCW, :, 2] = xp[:, 1:RT, :TCW].transpose(1, 2, 0)
    it[:RT - 1, :TCW, :, 3] = xp[:, 1:RT, 1:TCW + 1].transpose(1, 2, 0)
    return it.reshape(NIT, 256)


def host_prep(x, offset_weight, offset_bias, weight):
    """Returns list of 8 in_map dicts; core order = (b, hh)."""
    jj = np.arange(NJJ)
    kv = np.arange(KK)
    ki = kv // 3
    kj = kv % 3
    p = np.arange(128)

    # vertical-pair offset-conv weights: row u*64+c of pass kj = w[o, c, ki=u, kj]
    owr = offset_weight.reshape(27, C, 3, 3)
    ow2p2 = np.zeros((128, 3, 27), np.float16)
    for u in range(2):
        for q in range(3):
            ow2p2[u * 64:(u + 1) * 64, q, :] = owr[:, :, u, q].T
    ow2s2 = np.zeros((C, 3, 27), np.float16)
    for q in range(3):
        ow2s2[:, q, :] = owr[:, :, 2, q].T
    ob = offset_bias.reshape(27, 1).astype(np.float32)
    w2 = weight.reshape(O, C, KK)
    w2p = np.zeros((128, 4, 128), np.float16)
    for t in range(4):
        w2p[:64, t, :] = w2[:, :, 2 * t].T.astype(np.float16)
        w2p[64:, t, :] = w2[:, :, 2 * t + 1].T.astype(np.float16)
    w2s = np.ascontiguousarray(w2[:, :, 8].T.astype(np.float16))          # [64, 128]

    x0b = ((jj[None, :, None] % 2) * 128 + p[:, None, None] + kj[None, None, :] - 1
           ).astype(np.float32).reshape(128, NF)
    base = ((jj[None, :, None] // 2 + BD + ki[None, None, :]) * PITCH
            + (jj[None, :, None] % 2) * 128 + p[:, None, None]
            + kj[None, None, :] + MX).astype(np.float32)                   # [128, 64, 9]
    # BASEG [128, NQ, KK, JQ, 8] f32: [rr, Q, k, j, q] = base[q*16+rr, Q*16+j, k]
    baseg = np.zeros((16, NQ, KK, JQ, 8), np.float32)
    for q in range(8):
        for rr in range(16):
            b16 = base[q * 16 + rr]                      # [64, 9]
            baseg[rr, :, :, :, q] = b16.reshape(NQ, JQ, KK).transpose(0, 2, 1)
    baseg = np.tile(baseg, (8, 1, 1, 1, 1)).reshape(128, NQ * KK * JQ * 8)

    # E-phase fold matrices: matmul q maps varf partitions q*16..q*16+15 to a
    # [32, nf] PSUM tile with rows 16-31 duplicating rows 0-15, so the idx add
    # writes partitions 0-31 directly (the gather ucode reads exactly 32) and
    # no broadcast DMA is needed.
    permq = np.zeros((128, 8, 32), dtype=np.float32)
    for q in range(8):
        for rr in range(16):
            permq[q * 16 + rr, q, rr] = 1.0
            permq[q * 16 + rr, q, 16 + rr] = 1.0
    permq = permq.reshape(128, 256)
    idf16 = np.eye(128, dtype=np.float16)
    idf32 = np.eye(27, dtype=np.float32)
    # per-partition constants for 2-input DVE ops (1-input tensor_scalar/copy
    # ops run in DVE 2-port mode, which contends with the Q7 gather
    # descriptor-gen for SBUF ports and stalls up to 12us mid-stream):
    # cols: 0, 1, -1, H-1, H-2, W-1, W-2, round-magic (1.5*2^23)
    cb = np.tile(np.array([0.0, 1.0, -1.0, H - 1, H - 2, W - 1, W - 2,
                           12582912.0], np.float32), (128, 1))

    in_maps = []
    for core in range(8):
        b, hh = core // 2, core % 2
        h0 = hh * 32
        xp = _xpad_slice(x[b], h0)
        hoky = ((h0 + jj[None, :, None] // 2 + ki[None, None, :] - 1)
                * np.ones((128, 1, 1))).astype(np.float32)
        in_maps.append({
            "XP2": np.ascontiguousarray(_xp2(xp).reshape(128, RT * XCW)),
            "TBL": np.ascontiguousarray(_table(xp)),
            "OW2P2": np.ascontiguousarray(ow2p2.reshape(128, 3 * 27)),
            "OW2S2": np.ascontiguousarray(ow2s2.reshape(C, 3 * 27)),
            "OB": ob,
            "W2P": w2p, "W2S": w2s,
            "HOKY": np.ascontiguousarray(hoky.reshape(128, NF)), "X0B": x0b,
            "BASEG": baseg, "PERMQ": permq, "IDF16": idf16, "IDF32": idf32,
            "CB": cb,
        })
    return in_maps


def host_post(outs):
    """outs: list of 8 [128, 8192] f32 -> [4, 128, 64, 256]."""
    y = np.zeros((4, O, H, W), np.float32)
    for core, o in enumerate(outs):
        b, hh = core // 2, core % 2
        v = np.asarray(o).reshape(O, 32, 2, 128).reshape(O, 32, 256)
        y[b, :, hh * 32:hh * 32 + 32, :] = v
    return y


def _bcast(ap, dim, n):
    """Insert a [0, n] broadcast dim at free position `dim` (1-based in ap list)."""
    newap = [list(d) for d in ap.ap]
    newap.insert(dim, [0, n])
    return bass.AP(tensor=ap.tensor, offset=ap.offset, ap=newap)


def build(nc=None):
    if nc is None:
        nc = bacc.Bacc("TRN2", target_bir_lowering=False, debug=False)
    XP2 = nc.dram_tensor("XP2", [128, RT * XCW], F16, kind="ExternalInput")
    TBL = nc.dram_tensor("TBL", [NIT, 256], F16, kind="ExternalInput")
    OW2P2 = nc.dram_tensor("OW2P2", [128, 3 * 27], F16, kind="ExternalInput")
    OW2S2 = nc.dram_tensor("OW2S2", [C, 3 * 27], F16, kind="ExternalInput")
    OB = nc.dram_tensor("OB", [27, 1], F32, kind="ExternalInput")
    W2P = nc.dram_tensor("W2P", [128, 4, 128], F16, kind="ExternalInput")
    W2S = nc.dram_tensor("W2S", [C, 128], F16, kind="ExternalInput")
    HOKY = nc.dram_tensor("HOKY", [128, NF], F32, kind="ExternalInput")
    X0B = nc.dram_tensor("X0B", [128, NF], F32, kind="ExternalInput")
    BASEG = nc.dram_tensor("BASEG", [128, NQ * KK * JQ * 8], F32, kind="ExternalInput")
    PERMQ = nc.dram_tensor("PERMQ", [128, 8 * 32], F32, kind="ExternalInput")
    IDF16 = nc.dram_tensor("IDF16", [128, 128], F16, kind="ExternalInput")
    IDF32 = nc.dram_tensor("IDF32", [27, 27], F32, kind="ExternalInput")
    CB = nc.dram_tensor("CB", [128, 8], F32, kind="ExternalInput")
    OUT = nc.dram_tensor("OUT", [O, NPOS], F32, kind="ExternalOutput")

    mm = mybir.AluOpType

    with tile.TileContext(nc) as tc:
        with (
            tc.tile_pool(name="const", bufs=1) as cpool,
            tc.tile_pool(name="wq", bufs=1) as wq,
            tc.tile_pool(name="wq2", bufs=2) as wq2,
            tc.tile_pool(name="gat", bufs=3) as gpool,
            tc.tile_pool(name="hb", bufs=2) as hpool,
            tc.tile_pool(name="samp", bufs=2) as spool,
            tc.tile_pool(name="rhs", bufs=12) as rpool,
            tc.tile_pool(name="rhss", bufs=5) as rspool,
            tc.tile_pool(name="oq", bufs=2) as opool,
            tc.tile_pool(name="psB", bufs=1, space="PSUM") as psB,
            tc.tile_pool(name="psP", bufs=1, space="PSUM") as psP,
            tc.tile_pool(name="psTp", bufs=2, space="PSUM") as psTp,
            tc.tile_pool(name="psO", bufs=4, space="PSUM") as psO,
        ):
            # ---------- constants, ordered by when the serial prep(0) chain
            # needs them: tiny offset-conv weights, then just enough xp2 rows
            # for the first slabs' conv; the bulky baseg/xp2-tail/w2p queue
            # after everything that gates the first gather ----------
            ow2p2 = cpool.tile([128, 3, 27], F16)
            nc.sync.dma_start(out=ow2p2,
                              in_=OW2P2[:, :].rearrange("c (k o) -> c k o", k=3))
            ow2s2 = cpool.tile([C, 3, 27], F16)
            nc.sync.dma_start(out=ow2s2,
                              in_=OW2S2[:, :].rearrange("c (k o) -> c k o", k=3))
            ob = cpool.tile([27, 1], F32)
            nc.sync.dma_start(out=ob, in_=OB[:, :])
            idf32 = cpool.tile([27, 27], F32)
            nc.sync.dma_start(out=idf32, in_=IDF32[:, :])
            cb = cpool.tile([128, 8], F32)
            nc.sync.dma_start(out=cb, in_=CB[:, :])
            xp2 = cpool.tile([128, RT, XCW], F16)
            nc.sync.dma_start(out=xp2[:, 0:16, :],
                              in_=XP2[:, 0:16 * XCW].rearrange(
                                  "c (r w) -> c r w", r=16))

            def cbv(col, n, p=128):
                """[p, n] broadcast AP of constant column `col` of cb."""
                a = cb[0:p, col:col + 1]
                return bass.AP(tensor=a.tensor, offset=a.offset,
                               ap=[list(a.ap[0]), [0, n]])
            hoky = cpool.tile([128, NF], F32)
            nc.sync.dma_start(out=hoky, in_=HOKY[:, :])
            x0b = cpool.tile([128, NF], F32)
            nc.sync.dma_start(out=x0b, in_=X0B[:, :])
            permq = cpool.tile([128, 8, 32], F32)
            nc.sync.dma_start(out=permq,
                              in_=PERMQ[:, :].rearrange("c (q m) -> c q m", q=8))
            baseg = cpool.tile([128, NQ, KK, JQ, 8], F32)
            nc.sync.dma_start(out=baseg,
                              in_=BASEG[:, :].rearrange(
                                  "r (a k j q) -> r a k j q", a=NQ, k=KK, j=JQ))
            nc.sync.dma_start(out=xp2[:, 16:RT, :],
                              in_=XP2[:, 16 * XCW:].rearrange(
                                  "c (r w) -> c r w", r=RT - 16))
            w2p = cpool.tile([128, 4, 128], F16)
            nc.sync.dma_start(out=w2p, in_=W2P[:, :, :])
            w2s = cpool.tile([C, 128], F16)
            nc.sync.dma_start(out=w2s, in_=W2S[:, :])
            idf16 = cpool.tile([128, 128], F16)
            nc.sync.dma_start(out=idf16, in_=IDF16[:, :])

            def prep_quarter(jj0, njj):
                """Phases B-E for a slab of njj jj-blocks starting at jj0.

                Returns (v4_q, idxg_q) double-buffered per-slab tiles so the
                next slab's prep never WAR-stalls on this slab's gathers.
                The first quarter runs as two 8-jj slabs to halve the serial
                prep latency ahead of the first gather."""
                nf = njj * KK
                s = slice(jj0 * KK, (jj0 + njj) * KK)   # slice in (jj,k) layout
                Q = jj0 // JQ
                j0l = jj0 % JQ

                # ---- B: offset conv (vertical tap pairs; 6 passes) ----
                om_q = wq.tile([27, njj * 128], F32, tag="om")
                for cc in range(njj // 4):
                    ch = jj0 // 4 + cc
                    ps = psB.tile([27, 512], F32, tag="psom")
                    r = 2 * ch + BD
                    for kj in range(3):
                        rhs = xp2[:, r:r + 2, kj + MX:kj + MX + W]
                        nc.tensor.matmul(ps, ow2p2[:, kj, :], rhs,
                                         start=(kj == 0), stop=False)
                    for kj in range(3):
                        rhs = xp2[0:C, r + 2:r + 4, kj + MX:kj + MX + W]
                        nc.tensor.matmul(ps, ow2s2[:, kj, :], rhs,
                                         start=False, stop=(kj == 2))
                    nc.vector.scalar_tensor_tensor(
                        out=om_q[:, cc * 512:(cc + 1) * 512], in0=ps,
                        scalar=ob, in1=cbv(0, 512, p=27),
                        op0=mm.add, op1=mm.add)

                # ---- C: transpose om -> omt_q [128, 16, 27] ----
                omt_q = wq.tile([128, njj, 27], F32, tag="omt")
                for j in range(njj):
                    pst = psP.tile([128, 27], F32, tag="prep_ps")
                    nc.tensor.transpose(pst, om_q[:, j * 128:(j + 1) * 128], idf32)
                    nc.scalar.copy(omt_q[:, j, :], pst)

                # ---- D: weight math on [128, 144] ----
                DY = omt_q[:, :, 0:18:2]
                DX = omt_q[:, :, 1:18:2]
                MZ = omt_q[:, :, 18:27]

                msig = wq.tile([128, nf], F32, tag="msig")
                nc.scalar.activation(out=msig, in_=MZ,
                                     func=mybir.ActivationFunctionType.Sigmoid)

                def floor_frac(src_ap, ftag):
                    # floor via fp32 magic rounding (t = (src+1.5*2^23)-1.5*2^23
                    # rounds to integer; then f = t - (src < t)). All 2-input
                    # TT ops: no DVE 2-port mode, no Q7 port contention.
                    tm = wq.tile([128, nf], F32, tag="flr_m")
                    nc.vector.tensor_tensor(out=tm, in0=src_ap,
                                            in1=cbv(7, nf), op=mm.add)
                    tf = wq.tile([128, nf], F32, tag="flr_f")
                    nc.vector.tensor_tensor(out=tf, in0=tm,
                                            in1=cbv(7, nf), op=mm.subtract)
                    lt = wq.tile([128, nf], F32, tag="flr_lt")
                    nc.vector.tensor_tensor(out=lt, in0=src_ap, in1=tf,
                                            op=mm.is_lt)
                    fl = wq.tile([128, nf], F32, tag=ftag + "_f")
                    nc.vector.tensor_tensor(out=fl, in0=tf, in1=lt,
                                            op=mm.subtract)
                    fr = wq.tile([128, nf], F32, tag=ftag + "_l")
                    nc.vector.tensor_tensor(out=fr, in0=src_ap, in1=fl,
                                            op=mm.subtract)
                    return fl, fr

                fy, ly = floor_frac(DY, "fy")
                fx, lx = floor_frac(DX, "fx")

                y0 = wq.tile([128, nf], F32, tag="y0")
                nc.vector.tensor_tensor(out=y0, in0=fy, in1=hoky[:, s], op=mm.add)
                x0 = wq.tile([128, nf], F32, tag="x0")
                nc.vector.tensor_tensor(out=x0, in0=fx, in1=x0b[:, s], op=mm.add)

                def in_range(src, lo_col, hi_col, out_tag):
                    a = wq.tile([128, nf], F32, tag="rng_a")
                    nc.vector.tensor_tensor(out=a, in0=src,
                                            in1=cbv(lo_col, nf), op=mm.is_ge)
                    bq = wq.tile([128, nf], F32, tag="rng_b")
                    nc.vector.tensor_tensor(out=bq, in0=src,
                                            in1=cbv(hi_col, nf), op=mm.is_le)
                    rr = wq.tile([128, nf], F32, tag=out_tag)
                    nc.vector.tensor_tensor(out=rr, in0=a, in1=bq, op=mm.mult)
                    return rr

                vy0 = in_range(y0, 0, 3, "vy0")
                vy1 = in_range(y0, 2, 4, "vy1")
                vx0 = in_range(x0, 0, 5, "vx0")
                vx1 = in_range(x0, 2, 6, "vx1")

                ily = wq.tile([128, nf], F32, tag="ily")
                nc.vector.scalar_tensor_tensor(out=ily, in0=ly, scalar=-1.0,
                                               in1=cbv(1, nf),
                                               op0=mm.mult, op1=mm.add)
                ilx = wq.tile([128, nf], F32, tag="ilx")
                nc.vector.scalar_tensor_tensor(out=ilx, in0=lx, scalar=-1.0,
                                               in1=cbv(1, nf),
                                               op0=mm.mult, op1=mm.add)

                a0 = wq.tile([128, nf], F32, tag="a0")
                nc.vector.tensor_tensor(out=a0, in0=msig, in1=ily, op=mm.mult)
                nc.vector.tensor_tensor(out=a0, in0=a0, in1=vy0, op=mm.mult)
                a1 = wq.tile([128, nf], F32, tag="a1")
                nc.vector.tensor_tensor(out=a1, in0=msig, in1=ly, op=mm.mult)
                nc.vector.tensor_tensor(out=a1, in0=a1, in1=vy1, op=mm.mult)
                b0 = wq.tile([128, nf], F32, tag="b0")
                nc.vector.tensor_tensor(out=b0, in0=ilx, in1=vx0, op=mm.mult)
                b1 = wq.tile([128, nf], F32, tag="b1")
                nc.vector.tensor_tensor(out=b1, in0=lx, in1=vx1, op=mm.mult)

                v4_q = wq2.tile([128, njj, KK, 4], F16, tag="v4")
                nc.vector.tensor_tensor(out=v4_q[:, :, :, 0], in0=a0, in1=b0, op=mm.mult)
                nc.vector.tensor_tensor(out=v4_q[:, :, :, 1], in0=a0, in1=b1, op=mm.mult)
                nc.vector.tensor_tensor(out=v4_q[:, :, :, 2], in0=a1, in1=b0, op=mm.mult)
                nc.vector.tensor_tensor(out=v4_q[:, :, :, 3], in0=a1, in1=b1, op=mm.mult)

                varf = wq.tile([128, nf], F32, tag="varf")
                nc.vector.scalar_tensor_tensor(out=varf, in0=fy, scalar=float(PITCH),
                                               in1=fx, op0=mm.mult, op1=mm.add)

                # ---- E: idx fold (8 perm matmuls -> varg [32,(k,j,q)]).
                # permq duplicates rows into partitions 16-31, covering both
                # Q7 core idx reads with no broadcast DMA. ----
                varg = wq.tile([32, KK, njj, 8], F32, tag="varg")
                for q in range(8):
                    psf = psP.tile([32, nf], F32, tag="prep_ps")
                    nc.tensor.matmul(psf, permq[:, q, :],
                                     varf[:, :], start=True, stop=True)
                    # psf free = (j, k); dst (rr, k, j, q)
                    src = bass.AP(tensor=psf.tensor, offset=psf[:, 0].offset,
                                  ap=[list(psf[:, :].ap[0]), [1, KK], [KK, njj]])
                    dst = bass.AP(tensor=varg.tensor,
                                  offset=varg[0, 0, 0, q].offset,
                                  ap=[[varg[:, 0, 0, 0].ap[0][0], 32],
                                      [njj * 8, KK], [8, njj]])
                    nc.scalar.copy(dst, src)

                idxg_q = wq2.tile([32, KK, njj, 8], I16, tag="idxg")
                nc.vector.tensor_tensor(out=idxg_q[:, :, :, :],
                                        in0=varg[:, :, :, :],
                                        in1=baseg[0:32, Q, :, j0l:j0l + njj, :],
                                        op=mm.add)
                return v4_q, idxg_q

            def transpose_pairs(sampt, pso_p, ts, nblk):
                """Transpose tap-pairs ts (ready once tap 2t+1 is computed) for
                all jl into [128, 512] f16 rhs tiles, then immediately fold
                them into the per-block PSUM accumulation (start on t==0) so
                only the single-tap work remains after the last gather."""
                for t in ts:
                    for blk in range(nblk):
                        rhs_t = rpool.tile([128, 512], F16, tag="rhs")
                        for i in range(4):
                            jl = 4 * blk + i
                            pstp = psTp.tile([128, 128], F16, tag="pstp")
                            nc.tensor.transpose(
                                pstp, sampt[:, jl, 2 * t:2 * t + 2, :], idf16)
                            nc.scalar.copy(rhs_t[:, i * 128:(i + 1) * 128],
                                           pstp)
                        if t == 0:
                            pso_t = psO.tile([O, 512], F32, tag="pso")
                            pso_p[blk] = pso_t
                        nc.tensor.matmul(pso_p[blk], w2p[:, t, :], rhs_t,
                                         start=(t == 0), stop=False)

            def finish_quarter(sampt, pso_p, jj0, nblk):
                """Single-tap transposes + final accumulating matmul + output."""
                for blk in range(nblk):
                    rhs_s = rspool.tile([C, 512], F16, tag="rhss")
                    for i in range(4):
                        jl = 4 * blk + i
                        psts = psTp.tile([C, 128], F16, tag="pstp")
                        nc.tensor.transpose(psts, sampt[:, jl, 8, :], idf16)
                        nc.scalar.copy(rhs_s[:, i * 128:(i + 1) * 128], psts)
                    nc.tensor.matmul(pso_p[blk], w2s, rhs_s,
                                     start=False, stop=True)
                    oq = opool.tile([O, 512], F32)
                    nc.scalar.copy(oq, pso_p[blk])
                    nc.sync.dma_start(out=OUT[:, jj0 * 128 + blk * 512:
                                              jj0 * 128 + (blk + 1) * 512],
                                      in_=oq)

            # ---------- software-pipelined main loop ----------
            # prep is issued one slab ahead so its DVE/PE ops never queue
            # behind the current slab's tap work in the strict-FIFO engine
            # queues; gathers for the next slab start with zero boundary
            # bubble. The first quarter runs as two 8-jj slabs so the first
            # gather issues after ~half the serial prep latency.
            slabs = [(0, 4), (4, 4), (8, 8), (16, 16), (32, 16),
                     (48, 8), (56, 8)]
            vi = prep_quarter(*slabs[0])
            for si, (jj0, njj) in enumerate(slabs):
                vi_next = (prep_quarter(*slabs[si + 1])
                           if si + 1 < len(slabs) else None)
                v4_q, idxg_q = vi
                nblk = njj // 4
                sampt = spool.tile([128, JQ, KK, C], F16)
                pso_p = [None] * nblk
                for k in range(KK):
                    g = gpool.tile([128, JQ, 256], F16)
                    nc.gpsimd.dma_gather(
                        out_ap=g[:, 0:njj, :],
                        in_ap=TBL[:, :],
                        idxs_ap=idxg_q[:, k, :, :],
                        num_idxs=njj * 128,
                        num_idxs_reg=njj * 128,
                        elem_size=256, single_packet=False,
                    )
                    v4s = v4_q[:, :, k, :]
                    v4v = _bcast(v4s, 2, C)
                    h = hpool.tile([128, JQ, C, 4], F16, tag="h")
                    nc.vector.tensor_tensor(
                        out=h[:, 0:njj, :, :],
                        in0=g[:, 0:njj, :].rearrange("p j (c f) -> p j c f", c=C),
                        in1=v4v, op=mm.mult)
                    # 4-corner sum as two adds: first runs in DVE 2x mode
                    # (packed fp16 pairs), beats tensor_reduce's 1x.
                    s2 = hpool.tile([128, JQ, C, 2], F16, tag="s2")
                    with nc.allow_low_precision(reason="4-corner fp16 sum"):
                        nc.vector.tensor_tensor(
                            out=s2[:, 0:njj, :, :], in0=h[:, 0:njj, :, 0:2],
                            in1=h[:, 0:njj, :, 2:4], op=mm.add)
                        nc.vector.tensor_tensor(
                            out=sampt[:, 0:njj, k, :], in0=s2[:, 0:njj, :, 0],
                            in1=s2[:, 0:njj, :, 1], op=mm.add)
                    if k == 3:
                        transpose_pairs(sampt, pso_p, (0, 1), nblk)
                    if k == 7:
                        transpose_pairs(sampt, pso_p, (2, 3), nblk)
                finish_quarter(sampt, pso_p, jj0, nblk)
                vi = vi_next
    nc.compile()
    return nc


# ======================= runner =======================
_NC = None


def _get_nc():
    global _NC
    if _NC is None:
        _NC = build()
    return _NC


def kernel(x, offset_weight, offset_bias, weight):
    from concourse.bass_utils import run_bass_kernel_spmd
    x = np.asarray(x)
    offset_weight = np.asarray(offset_weight)
    offset_bias = np.asarray(offset_bias)
    weight = np.asarray(weight)
    in_maps = host_prep(x, offset_weight, offset_bias, weight)
    nc = _get_nc()
    res = run_bass_kernel_spmd(nc, in_maps, core_ids=list(range(8)))
    return host_post([r["OUT"] for r in res.results])
